# revision 51
# baseline (speedup 1.0000x reference)
"""CodaPrompt kernel for Trainium2 (Bass/Tile) on 8 NeuronCores.

Math (reference):
    a[e,b,k,:] = x[b,:] * As[e,k,:]
    q = a / max(||a||_2, eps)        (normalize over d)
    nK = Ks / max(||Ks||_2, eps)
    aq[e,b,k] = <q[e,b,k,:], nK[e,k,:]>
    P_[e,b,l,:] = sum_k aq[e,b,k] * Ps[e,k,l,:]
    out = stack([P_[:,:, :L/2], P_[:,:, L/2:]])   # [2, E, B, L/2, D]

Sharding: SSPLIT L-slices x (8/SSPLIT) batch-slices (default 4x2). Splitting
the output L-axis (the stack axis) cuts each core's Ps load to 1/SSPLIT vs
pure batch-parallel at identical arithmetic — the kernel is DMA-bound and
the output (31.5MB/core) is fixed, so input bytes are the only lever
(default config: 3.1MB Ps + 3.1MB x + 3.1MB weights vs 16.2MB for pure
batch-parallel). Each core computes the full cosine-weight stage (aq) for
its batch slice (duplicated across L-slices; PE has slack).

Device-side formulation (per core: batch slice of BC rows, one L-slice):
    num[e,k,b] = sum_d (As*nK)[e,k,d] * x[b,d]        -> matmul, contraction over d
    den2[e,k,b] = sum_d (As*As)[e,k,d] * x2[b,d]      -> matmul (x2 on device)
    aq[e,k,b] = num * rsqrt(den2)                      (ACT sqrt + DVE recip + mul)
    P_half[b, (l d)] = aq[e,:,b].T @ Ps[e, :, half]    -> matmul, contraction over k

Host prep is O(E*K*D) pool preprocessing (normalize Ks, fuse/transpose
weights, slice Ps halves) plus the x transpose; all O(B*...) FLOPs on device.
"""

import os
import sys
from contextlib import ExitStack

import numpy as np

if "/opt/trn_rl_repo" not in sys.path:
    sys.path.insert(0, "/opt/trn_rl_repo")

import concourse.mybir as mybir
from concourse import bacc, tile
from concourse.bass_utils import run_bass_kernel_spmd

B, D, E, K, L = 2048, 768, 5, 100, 8
NCORES = 8
SSPLIT = int(os.environ.get("CODA_SSPLIT", "4"))  # L-axis splits (2 or 4)
QSPLIT = NCORES // SSPLIT # batch splits
BC = B // QSPLIT          # batch rows per core
LH = L // SSPLIT          # l entries per core
DC = D // 128             # 6 contraction chunks of 128
KP = 128                  # pool axis padded so per-(e,c) W DMAs have 512B runs
NDH = LH * D              # P_ cols per core
NCHUNK = 512              # psum bank width in f32
NJ = NDH // NCHUNK        # n-chunks per core
MC = BC // 128            # output-partition chunks
NB = max(1, BC // 512)    # moving-operand chunks for num/den (fp32 N<=512)
EPS = 1e-12

F32 = mybir.dt.float32
F8 = mybir.dt.float8e4
# The kernel is DMA-bound (the fp32 variant sits at the 360 GB/s roofline),
# so all wire traffic defaults to bf16: ~0.4% per-element quantization vs the
# 2e-2 gate, and it halves both the input loads and the 15.7MB/core output
# store. "float32r" / "float32" remain available via env for A/B checks.
MM_DTYPE = os.environ.get("CODA_MM_DTYPE", "bfloat16")
MM_DT = getattr(mybir.dt, MM_DTYPE)
PS_DTYPE = os.environ.get("CODA_PS_DTYPE", MM_DTYPE)
PS_DT = getattr(mybir.dt, PS_DTYPE)
# Output wire dtype: psum (always f32) is cast during the psum->sbuf copy,
# stored at this dtype, and upcast to f32 on the host.
OUT_DTYPE = os.environ.get("CODA_OUT_DTYPE", "bfloat16")
OUT_DT = getattr(mybir.dt, OUT_DTYPE)


def _build_bass(repeat=1):
    # Bacc (not plain Bass): its finalize() runs move_matmul_waits_to_ldweights
    # + generate_event_semaphores, without which multi-dependency matmuls hit
    # walrus "Too many sync wait commands".
    # `repeat` replicates the whole compute body (timing instrumentation:
    # slope over repeat removes per-launch overhead); results are idempotent.
    nc = bacc.Bacc(None)

    # Matmul operands must be produced as MM_DT end-to-end (walrus verifies
    # fp32r consumers see fp32r producers). float32r is bit-identical to
    # float32 in DRAM, so host arrays stay np.float32 either way.
    # W is laid out per-e ([E, D, 2, KP], K padded to KP=128 so each (e,c)
    # DMA slice has 512B descriptor runs): e0's weights finish loading right
    # after x, so the first stores issue at ~9us instead of queueing behind
    # the full weight/pool load stream (~17us) — the psum->sbuf copy engines
    # (ACT+DVE, ~92us of combined work) need that whole span.
    xT_d = nc.declare_dram_parameter("xT", [D, BC], MM_DT, isOutput=False)
    w_d = nc.declare_dram_parameter("w12T", [E, D, 2, KP], MM_DT, isOutput=False)
    ps_d = nc.declare_dram_parameter("ps", [E, K, NDH], PS_DT, isOutput=False)
    out_d = nc.declare_dram_parameter("out", [E, BC, LH, D], OUT_DT, isOutput=True)

    with ExitStack() as ctx:
        tc = ctx.enter_context(tile.TileContext(nc))
        const = ctx.enter_context(tc.tile_pool(name="const", bufs=1))
        psp = ctx.enter_context(tc.tile_pool(name="psp", bufs=E))
        smallp = ctx.enter_context(tc.tile_pool(name="smallp", bufs=2))
        resp = ctx.enter_context(tc.tile_pool(name="resp", bufs=4))
        # PSUM budget (8 banks of 2KB): num+den half-batch tiles [K,512] are
        # 1 bank each (2 total, bufs=1 — h1 reusing h0's bank only waits on
        # h0's aq-mul, already pipelined ~2 blocks earlier), ppA 2x[128,1024]
        # = 4 banks, ppB 2x[128,512] = 2 banks.
        pndp = ctx.enter_context(tc.tile_pool(name="pndp", bufs=1, space="PSUM"))
        ppap = ctx.enter_context(tc.tile_pool(name="ppap", bufs=2, space="PSUM"))
        ppbp = ctx.enter_context(tc.tile_pool(name="ppbp", bufs=2, space="PSUM"))

        # Resident operands: x quarter (transposed) and the fused W1=As*nK /
        # W2=As^2 weight block, chunked to 128 partitions. Per-chunk loads so
        # the first num/den matmuls start as soon as their own d-chunk lands.
        # x^2 is computed on-device (saves its DMA).
        xT_r = xT_d[:].rearrange("(c p) b -> p c b", p=128)
        w_r = w_d[:].rearrange("e (c p) t k -> p e c t k", p=128)
        xs = const.tile([128, DC, BC], MM_DT, name="xs", tag="xs")
        ws = const.tile([128, E, DC, 2, KP], MM_DT, name="ws", tag="ws")
        # The den bilinear form runs in fp8e4 DoubleRow mode (0.5 cycles/row,
        # two contraction chunks per matmul): den only feeds rsqrt, and its
        # all-positive d-sum suppresses the ~6% fp8 quantization to <1%. Both
        # fp8 operands are produced ON DEVICE (x^2 by DVE with an fp8 output,
        # W2 casts by the mostly-idle Pool engine), so no DMA layout changes.
        # (ws2f8's k dim stays padded to KP: DoubleRow ldweights requires the
        # dual-row stride to be a multiple of 16 bytes — K=100 is not.)
        x2s = const.tile([128, DC, BC], F8, name="x2s", tag="x2s")
        ws2f8 = const.tile([128, E, DC, KP], F8, name="ws2f8", tag="ws2f8")
        # Interleaved per-chunk loads, x leading w: the first num/den matmuls
        # chase the load stream chunk by chunk (keeping PE warm through its
        # p-state ramp), and each x^2 (DVE, idle during loads) lands while
        # the NEXT w chunk is still in flight — an x-then-w or w-then-x bulk
        # order leaves den(h0) gated ~2us later either by a cold-PE matmul
        # burst or by the x^2 tail. (x2 on GpSimd: 0.42 multiply efficiency
        # made it a serial 13us startup phase.)
        # Bootstrap load order: x and e0's weights interleaved per chunk (x
        # leading w: the first num/den matmuls chase the load stream while
        # each x^2 on otherwise-idle DVE lands during the next w chunk's
        # flight), then e0's prompt pool, then the remaining weights and
        # pools. Everything e0 needs is resident by ~7us, so the first
        # stores issue while later weights are still loading.
        for c in range(DC):
            nc.sync.dma_start(xs[:, c], xT_r[:, c])
            nc.sync.dma_start(ws[:, 0, c], w_r[:, 0, c])
            nc.vector.tensor_mul(x2s[:, c], xs[:, c], xs[:, c])
            nc.gpsimd.tensor_copy(ws2f8[:, 0, c, :K], ws[:, 0, c, 1, :K])

        for _ in range(repeat):
            # Pool loads get their own slots (bufs=E); stores ride the Pool
            # engine's DMA queue (see below) so no store ever queues behind
            # these later loads in the SP DMA FIFO.
            psts = []
            for e in range(E):
                pst = psp.tile([K, NDH], PS_DT, name="pst", tag="ps")
                psts.append(pst)
            nc.sync.dma_start(psts[0][:], ps_d[0])
            for e in range(1, E):
                for c in range(DC):
                    nc.sync.dma_start(ws[:, e, c], w_r[:, e, c])
                nc.sync.dma_start(psts[e][:], ps_d[e])
            # Software pipeline over e: the num/den accumulation groups and
            # the aq chain for e+1 are interleaved between the P_ m-blocks of
            # e. Without this, each e->e+1 transition serializes ~5us of PE
            # num/den behind all of e's P_ matmuls while sqrt(e+1) head-blocks
            # the ACT FIFO — the copy engines (and thus the output stores)
            # starve and the DMA roofline is lost to ~1.5us gaps per
            # transition plus a ~7us startup bubble.

            def emit_nd_group(st, e, nb, t):
                # One accumulation group: the half-batch den (fp8 DoubleRow,
                # 3 paired-chunk matmuls) or num (bf16, 6 chunk matmuls).
                key = ("num", "den")[t]
                dst = pndp.tile([K, 512], F32, name=key, tag=key)
                st[key] = dst
                bsl = slice(nb * 512, (nb + 1) * 512)
                if t == 1:
                    for cc in range(0, DC, 2):
                        nc.tensor.matmul(
                            dst[:],
                            ws2f8[:, e, cc : cc + 2, :K],
                            x2s[:, cc : cc + 2, bsl],
                            start=(cc == 0),
                            stop=(cc == DC - 2),
                            perf_mode=mybir.MatmulPerfMode.DoubleRow,
                        )
                else:
                    for c in range(DC):
                        nc.tensor.matmul(
                            dst[:],
                            ws[:, e, c, 0, :K],
                            xs[:, c, bsl],
                            start=(c == 0),
                            stop=(c == DC - 1),
                        )

            def emit_aq_half(st, nb):
                # aq = num / sqrt(den2) (den2 >> eps^2 here), per b-half so
                # P_ m-blocks of the first half never wait on the second.
                # Chain engines: sqrt ACT, reciprocal + aq-mul DVE
                # (reciprocal is DVE-only; GpSimd can't run it or touch the
                # PSUM-resident num).
                bsl = slice(nb * 512, (nb + 1) * 512)
                sden = smallp.tile([K, 512], F32, name="sden", tag="sden")
                rden = smallp.tile([K, 512], F32, name="rden", tag="rden")
                nc.scalar.sqrt(sden[:], st["den"][:])
                nc.vector.reciprocal(rden[:], sden[:])
                nc.vector.tensor_mul(st["aq"][:, bsl], st["num"][:], rden[:])

            def emit_w2_casts(e1, cs):
                for c in cs:
                    nc.gpsimd.tensor_copy(ws2f8[:, e1, c, :K], ws[:, e1, c, 1, :K])

            def next_e_pieces(e1):
                # Deferred thunks, spread over the P_ m-blocks of earlier e.
                # W2 fp8 casts lead (den needs them); each half's den/num is
                # followed directly by its aq so the 1-bank psum slots
                # recycle between halves. den before num: its chain (sqrt,
                # recip) overlaps num's matmuls.
                st1 = {"aq": smallp.tile([K, BC], PS_DT, name="aq", tag="aq", bufs=2)}
                states[e1] = st1
                pieces = []
                if e1 > 0:  # e0's casts are inline in the bootstrap loop
                    pieces.append(lambda: emit_w2_casts(e1, range(0, DC, 2)))
                    pieces.append(lambda: emit_w2_casts(e1, range(1, DC, 2)))
                for nb in range(NB):
                    pieces.append(lambda nb=nb: emit_nd_group(st1, e1, nb, 1))
                    pieces.append(lambda nb=nb: emit_nd_group(st1, e1, nb, 0))
                    pieces.append(lambda nb=nb: emit_aq_half(st1, nb))
                return pieces

            states = {}
            pending = []
            p0 = next_e_pieces(0)
            # Only e0's first-half chain goes inline (it gates the first
            # store); everything else rides the m-block piece slots.
            for piece in p0[:3]:
                piece()
            pending.extend(p0[3:])

            for e in range(E):
                aq = states.pop(e)["aq"]
                pst = psts[e]
                if e + 1 < E:
                    pending.extend(next_e_pieces(e + 1))
                for m in range(MC):
                    # Pipeline pieces ride the m-blocks starting at m=1 (m=0
                    # lets this e's first store issue first); normally one per
                    # block, two only when backlogged (e0 carries 9).
                    if m >= 1 and pending:
                        npop = min(2, max(1, -(-len(pending) // (MC - m))))
                        for piece in pending[:npop]:
                            piece()
                        del pending[:npop]
                    if m % 2 == 0:
                        # One res tile and one store per TWO m-blocks
                        # (DRAM-contiguous in b): halves the per-store SWDGE
                        # descriptor-gen overhead (~1us of Pool engine each)
                        # that otherwise jitters the store pacing.
                        res2 = resp.tile([128, 2 * NDH], OUT_DT, name="res", tag="res")
                    res = res2[:, (m % 2) * NDH : (m % 2 + 1) * NDH]
                    aqm = aq[:, m * 128 : (m + 1) * 128]
                    ppa = ppap.tile([128, 1024], F32, name="ppa", tag="ppa")
                    for j in range(2):
                        nc.tensor.matmul(
                            ppa[:, j * NCHUNK : (j + 1) * NCHUNK],
                            aqm,
                            pst[:, j * NCHUNK : (j + 1) * NCHUNK],
                            start=True,
                            stop=True,
                        )
                    # Fixed copy assignment — A (1024) on ACT, B (512) on
                    # DVE: only DVE/ACT can read PSUM (walrus rejects
                    # GpSimd<->PSUM); every block's two copies run in
                    # parallel, pacing one store per ~1.0us; DVE's ~0.4us/
                    # block slack absorbs its recip/aq-mul/x^2 bursts.
                    # (ACT ~48us, DVE ~39us busy, both < 56.6us DMA.)
                    nc.scalar.copy(res[:, :1024], ppa[:])
                    ppb = ppbp.tile([128, NCHUNK], F32, name="ppb", tag="ppb")
                    nc.tensor.matmul(
                        ppb[:],
                        aqm,
                        pst[:, 2 * NCHUNK : 3 * NCHUNK],
                        start=True,
                        stop=True,
                    )
                    nc.vector.tensor_copy(res[:, 1024:], ppb[:])
                    # Stores go out on the (otherwise idle) Pool engine's DMA
                    # queue: the SP queue still holds the e1..e4 weight/pool
                    # loads when the first stores become ready, and a FIFO
                    # queue would serialize them behind it.
                    if (e == 0 and m < 2) or (e == E - 1 and m >= MC - 2):
                        # Finest-grained first/last stores: each half streams
                        # out as soon as its copy lands — pulls the DMA
                        # stream forward at the load->store handoff and
                        # shortens the final drain tail.
                        out_ap = out_d[e, m * 128 : (m + 1) * 128, :, :].rearrange(
                            "b l d -> b (l d)"
                        )
                        nc.gpsimd.dma_start(out_ap[:, :1024], res[:, :1024])
                        nc.gpsimd.dma_start(out_ap[:, 1024:], res[:, 1024:])
                    elif m % 2 == 1:
                        out2_ap = out_d[e, (m - 1) * 128 : (m + 1) * 128, :, :].rearrange(
                            "(mm p) l d -> p mm (l d)", p=128
                        )
                        nc.gpsimd.dma_start(
                            out2_ap, res2[:].rearrange("p (mm ld) -> p mm ld", mm=2)
                        )
            for piece in pending:  # safety: drain leftovers
                piece()
            del pending[:]

    if not nc.is_finalized():
        nc.finalize()
    return nc


_NC_CACHE = None


def _get_nc():
    global _NC_CACHE
    if _NC_CACHE is None:
        _NC_CACHE = _build_bass()
    return _NC_CACHE


def _prep_inputs(x, Ks, As, Ps):
    x = np.asarray(x, dtype=np.float32)
    Ks = np.asarray(Ks, dtype=np.float32)
    As = np.asarray(As, dtype=np.float32)
    Ps = np.asarray(Ps, dtype=np.float32)

    nrm = np.sqrt(np.sum(Ks * Ks, axis=-1, keepdims=True))
    nK = Ks / np.maximum(nrm, EPS)
    w12T = np.zeros((E, D, 2, KP), dtype=np.float32)
    w12T[:, :, 0, :K] = (As * nK).transpose(0, 2, 1)
    w12T[:, :, 1, :K] = (As * As).transpose(0, 2, 1)

    ps_np = mybir.dt.np(PS_DT)
    mm_np = mybir.dt.np(MM_DT)
    ps_slices = [
        np.ascontiguousarray(
            Ps[:, :, si * LH : (si + 1) * LH, :].reshape(E, K, NDH)
        ).astype(ps_np, copy=False)
        for si in range(SSPLIT)
    ]
    w12T = w12T.astype(mm_np, copy=False)
    xT = np.ascontiguousarray(x.T).astype(mm_np, copy=False)  # [D, B]

    in_maps = []
    for c in range(NCORES):
        si, q = divmod(c, QSPLIT)
        in_maps.append(
            {
                "xT": np.ascontiguousarray(xT[:, q * BC : (q + 1) * BC]),
                "w12T": w12T,
                "ps": ps_slices[si],
            }
        )
    return in_maps


def _run(x, Ks, As, Ps, trace=False, **spmd_kwargs):
    nc = _get_nc()
    in_maps = _prep_inputs(x, Ks, As, Ps)
    res = run_bass_kernel_spmd(nc, in_maps, list(range(NCORES)), trace=trace, **spmd_kwargs)
    out = np.empty((2, E, B, L // 2, D), dtype=np.float32)
    for c in range(NCORES):
        si, q = divmod(c, QSPLIT)
        s, lp = divmod(si * LH, L // 2)
        out[s, :, q * BC : (q + 1) * BC, lp : lp + LH] = np.asarray(
            res.results[c]["out"]
        ).astype(np.float32)
    return out, res


def kernel(x, Ks, As, Ps):
    out, _ = _run(x, Ks, As, Ps, trace=False)
    return out

